# revision 1
# baseline (speedup 1.0000x reference)
"""Trainium2 Bass kernel for CAttentionBlock (windowed multi-head attention x4 + LN).

Computation per batch image (one NeuronCore each, pure data parallel over B=8):
  window-partition (2x2 windows, N=4 tokens, C=256, NH=8 heads, d=32)
  x1 = LN1(rw + attn(rw, gw, gw))
  x2 = LN2(gw + attn(gw, bw, bw))
  x3 = LN3(bw + attn(bw, iw, iw))
  x4 = LN4(iw + attn(iw, gw, gw))
  out = concat([x1, x2, x3, x4], -1)  -> [64, 64, 1024]

Layout: windows on SBUF partitions (128 windows/tile, 8 tiles/core), tokens x
channels on the free dim ([128, 4*256]).  The tiny 4x4 attention is computed
with broadcast access patterns on the vector engine (optionally bf16 at 2x);
the scalar engine does exp, probability expansion and LN statistics (via
accum_out); gpsimd takes part of the add tree; no matmuls.
"""

import sys

for _p in ("/opt/trn_rl_repo",):
    if _p not in sys.path:
        sys.path.insert(0, _p)

import numpy as np

import bass_rust
import concourse.bass as bass
import concourse.tile as tile
from concourse import mybir
from concourse.bass_utils import run_bass_kernel_spmd

F32 = mybir.dt.float32
BF16 = mybir.dt.bfloat16
ALU = mybir.AluOpType
ACT = mybir.ActivationFunctionType

B, H, W, C = 8, 64, 64, 256
WS = 2
NH = 8
D = C // NH            # 32
NTOK = WS * WS         # 4
NW = (H // WS) * (W // WS)   # 1024 windows per image
NWI = H // WS          # 32 window rows
P = 128                # partitions / windows per tile
NTILES = NW // P       # 8
EPS = 1e-5
INV_SQRT_D = 1.0 / float(np.sqrt(D))

USE_BF16 = False       # bf16 products (DVE 2x) for QK scores and p*V;
                       # False keeps full f32 accuracy (rel err ~2e-06)

# (q_tensor_idx, kv_tensor_idx) per attention; tensors ordered r,g,b,ir
ATTNS = [(0, 1), (1, 2), (2, 3), (3, 1)]


def _ap(ref, offset_delta, dims):
    """Build an AP on ref's tensor at ref.offset + delta with explicit
    [step, count] dims (elements)."""
    return bass_rust.AP(ref.tensor, ref.offset + offset_delta, [list(d) for d in dims])


def build_kernel(stage=6, use_bf16=None):
    """stage: 1=loads, 2=+QK, 3=+softmax, 4=+AV, 5=+resid/LN, 6=+out-DMA (full)."""
    if use_bf16 is None:
        use_bf16 = USE_BF16
    nc = bass.Bass("TRN2", target_bir_lowering=False, debug=False)

    ins = {
        name: nc.dram_tensor(name, [H, W, C], F32, kind="ExternalInput")
        for name in ("r", "g", "b", "ir")
    }
    ln_params = []
    for a in range(4):
        wv = nc.dram_tensor(f"ln{a + 1}_w", [C], F32, kind="ExternalInput")
        bv = nc.dram_tensor(f"ln{a + 1}_b", [C], F32, kind="ExternalInput")
        ln_params.append((wv, bv))
    out = nc.dram_tensor("out", [H, W, 4 * C], F32, kind="ExternalOutput")

    in_aps = [ins[n].ap() for n in ("r", "g", "b", "ir")]
    out_ap = out.ap()
    pdt = BF16 if use_bf16 else F32

    with tile.TileContext(nc) as tc:
        with (
            tc.tile_pool(name="const", bufs=1) as pconst,
            tc.tile_pool(name="pin", bufs=2) as pin,
            tc.tile_pool(name="pbig", bufs=2) as pbig,
            tc.tile_pool(name="pmed", bufs=2) as pmed,
            tc.tile_pool(name="pscr", bufs=1) as pscr,
            tc.tile_pool(name="pout", bufs=2) as pout,
            tc.tile_pool(name="psmall", bufs=2) as psmall,
        ):
            # LN weight/bias replicated to all 128 partitions
            wreps, breps = [], []
            for a, (wv, bv) in enumerate(ln_params):
                wt = pconst.tile([P, C], F32, tag=f"wrep{a}")
                bt = pconst.tile([P, C], F32, tag=f"brep{a}")
                nc.sync.dma_start(out=wt[:], in_=_ap(wv.ap(), 0, [[0, P], [1, C]]))
                nc.sync.dma_start(out=bt[:], in_=_ap(bv.ap(), 0, [[0, P], [1, C]]))
                wreps.append(wt)
                breps.append(bt)

            for t in range(NTILES):
                # ---- load the 4 input tiles: [128 windows, 4 tok * 256 ch]
                tiles = []
                tiles_b = []
                for xi, name in enumerate(("r", "g", "b", "ir")):
                    tx = pin.tile([P, NTOK * C], F32, tag=f"in{name}")
                    txr = tx[:]
                    for qh in range(2):
                        # one DMA covers all 128 windows' (qh, qw, c) half:
                        # DRAM run (j, qw, c) is 16K elems contiguous; the
                        # SBUF side stays canonical [128, F] so Tile's
                        # dependency tracking sees the true partition range.
                        src = _ap(
                            in_aps[xi],
                            (8 * t + qh) * W * C,
                            [[2 * W * C, 4], [2 * C, NWI], [1, 2 * C]],
                        )
                        dst = _ap(txr, qh * 2 * C, [txr.ap[0], [1, 2 * C]])
                        nc.sync.dma_start(out=dst, in_=src)
                    tiles.append(tx)
                    if use_bf16 and stage >= 2:
                        txb = pin.tile([P, NTOK * C], BF16, tag=f"inb{name}")
                        nc.scalar.activation(
                            out=txb[:], in_=tx[:], func=ACT.Copy, bias=0.0, scale=1.0
                        )
                        tiles_b.append(txb)

                # per-tile output assembly: [128, (tok, attn, c)] = [128, 4096]
                xout_all = pout.tile([P, NTOK * 4 * C], F32, tag="xout_all")
                xar = xout_all[:]

                for a, (qi, ki) in enumerate(ATTNS):
                    if stage < 2:
                        continue
                    qr = tiles[qi][:]
                    kr = tiles[ki][:]
                    qpr = tiles_b[qi][:] if use_bf16 else qr
                    kpr = tiles_b[ki][:] if use_bf16 else kr

                    # ---- scores: prod[w,(q,k,c)] = Q[w,q,c] * K[w,k,c]
                    prod = pbig.tile([P, 16 * C], pdt, tag="prod")
                    pr = prod[:]
                    nc.vector.tensor_tensor(
                        out=_ap(pr, 0, [pr.ap[0], [4 * C, 4], [C, 4], [1, C]]),
                        in0=_ap(qpr, 0, [qpr.ap[0], [C, 4], [0, 4], [1, C]]),
                        in1=_ap(kpr, 0, [kpr.ap[0], [0, 4], [C, 4], [1, C]]),
                        op=ALU.mult,
                    )
                    # s[w,(q,k,h)] = sum_d prod
                    s = psmall.tile([P, 128], F32, tag="s")
                    sr = s[:]
                    nc.vector.reduce_sum(
                        out=_ap(sr, 0, [sr.ap[0], [32, 4], [8, 4], [1, 8]]),
                        in_=_ap(pr, 0, [pr.ap[0], [4 * C, 4], [C, 4], [D, 8], [1, D]]),
                        axis=mybir.AxisListType.X,
                    )
                    if stage < 3:
                        continue
                    # e = exp(s / sqrt(d))  (no max-subtraction: scores ~ N(0,1))
                    e = psmall.tile([P, 128], F32, tag="e")
                    nc.scalar.activation(
                        out=e[:], in_=s[:], func=ACT.Exp, bias=0.0, scale=INV_SQRT_D
                    )
                    # Z[w,(q,h)] = sum_k e ; rinv = 1/Z
                    z = psmall.tile([P, 32], F32, tag="z")
                    er = e[:]
                    nc.vector.reduce_sum(
                        out=z[:],
                        in_=_ap(er, 0, [er.ap[0], [32, 4], [1, 8], [8, 4]]),
                        axis=mybir.AxisListType.X,
                    )
                    rz = psmall.tile([P, 32], F32, tag="rz")
                    nc.vector.reciprocal(out=rz[:], in_=z[:])
                    # pn[w,(q,k,h)] = e * rinv (broadcast over k)
                    pn = psmall.tile([P, 128], F32, tag="pn")
                    rzr = rz[:]
                    nc.vector.tensor_tensor(
                        out=pn[:],
                        in0=e[:],
                        in1=_ap(rzr, 0, [rzr.ap[0], [8, 4], [0, 4], [1, 8]]),
                        op=ALU.mult,
                    )
                    if stage < 4:
                        continue
                    # ---- AV: prodv[w,(q,k,c)] = pn[w,q,k,h(c)] * KV[w,k,c]
                    prodv = pbig.tile([P, 16 * C], pdt, tag="prodv")
                    pvr = prodv[:]
                    pnr = pn[:]
                    if use_bf16:
                        # expand pn across head dim on ACT (1x there anyway),
                        # keeping both product inputs bf16 step-1 for DVE 2x
                        pexp = pbig.tile([P, 16 * C], BF16, tag="pexp")
                        nc.scalar.activation(
                            out=pexp[:],
                            in_=_ap(
                                pnr,
                                0,
                                [pnr.ap[0], [32, 4], [8, 4], [1, 8], [0, D]],
                            ),
                            func=ACT.Copy,
                            bias=0.0,
                            scale=1.0,
                        )
                        nc.vector.tensor_tensor(
                            out=_ap(
                                pvr, 0, [pvr.ap[0], [4 * C, 4], [C, 4], [D, 8], [1, D]]
                            ),
                            in0=pexp[:],
                            in1=_ap(kpr, 0, [kpr.ap[0], [0, 4], [C, 4], [D, 8], [1, D]]),
                            op=ALU.mult,
                        )
                    else:
                        nc.vector.tensor_tensor(
                            out=_ap(
                                pvr, 0, [pvr.ap[0], [4 * C, 4], [C, 4], [D, 8], [1, D]]
                            ),
                            in0=_ap(
                                pnr, 0, [pnr.ap[0], [32, 4], [8, 4], [1, 8], [0, D]]
                            ),
                            in1=_ap(kr, 0, [kr.ap[0], [0, 4], [C, 4], [D, 8], [1, D]]),
                            op=ALU.mult,
                        )
                    # k-sum + residual:
                    #   xres = ((p0+p1) + Q) + (p2+p3)   [DVE, DVE, POOL, POOL]
                    t01 = pmed.tile([P, NTOK * C], pdt, tag="t01")
                    t23 = pmed.tile([P, NTOK * C], pdt, tag="t23")
                    qk_dims = [pvr.ap[0], [4 * C, 4], [1, C]]
                    nc.vector.tensor_tensor(
                        out=t01[:],
                        in0=_ap(pvr, 0 * C, qk_dims),
                        in1=_ap(pvr, 1 * C, qk_dims),
                        op=ALU.add,
                    )
                    nc.gpsimd.tensor_tensor(
                        out=t23[:],
                        in0=_ap(pvr, 2 * C, qk_dims),
                        in1=_ap(pvr, 3 * C, qk_dims),
                        op=ALU.add,
                    )
                    if stage < 5:
                        continue
                    # ---- residual
                    u = pmed.tile([P, NTOK * C], F32, tag="u")
                    nc.gpsimd.tensor_tensor(out=u[:], in0=t01[:], in1=qr, op=ALU.add)
                    xres = pmed.tile([P, NTOK * C], F32, tag="xres")
                    nc.gpsimd.tensor_tensor(
                        out=xres[:], in0=u[:], in1=t23[:], op=ALU.add
                    )
                    # ---- LN stats (scalar engine accumulators)
                    msum = psmall.tile([P, NTOK], F32, tag="msum")
                    ssq = psmall.tile([P, NTOK], F32, tag="ssq")
                    scr = pscr.tile([P, NTOK * C], F32, tag="scr")
                    for q in range(NTOK):
                        qs = slice(q * C, (q + 1) * C)
                        nc.scalar.activation(
                            out=scr[:, qs],
                            in_=xres[:, qs],
                            func=ACT.Copy,
                            bias=0.0,
                            scale=1.0,
                            accum_out=msum[:, q : q + 1],
                        )
                        nc.scalar.activation(
                            out=scr[:, qs],
                            in_=xres[:, qs],
                            func=ACT.Square,
                            bias=0.0,
                            scale=1.0,
                            accum_out=ssq[:, q : q + 1],
                        )
                    mu = psmall.tile([P, NTOK], F32, tag="mu")
                    nc.vector.tensor_scalar(
                        out=mu[:], in0=msum[:], scalar1=1.0 / C, scalar2=None,
                        op0=ALU.mult,
                    )
                    ex2 = psmall.tile([P, NTOK], F32, tag="ex2")
                    nc.vector.tensor_scalar(
                        out=ex2[:], in0=ssq[:], scalar1=1.0 / C, scalar2=None,
                        op0=ALU.mult,
                    )
                    var = psmall.tile([P, NTOK], F32, tag="var")
                    nc.vector.tensor_tensor(
                        out=var[:], in0=mu[:], in1=mu[:], op=ALU.mult
                    )
                    nc.vector.tensor_tensor(
                        out=var[:], in0=ex2[:], in1=var[:], op=ALU.subtract
                    )
                    vpe = psmall.tile([P, NTOK], F32, tag="vpe")
                    nc.vector.tensor_scalar(
                        out=vpe[:], in0=var[:], scalar1=EPS, scalar2=None, op0=ALU.add
                    )
                    rinv = psmall.tile([P, NTOK], F32, tag="rinv")
                    nc.vector.reciprocal(out=rinv[:], in_=vpe[:])
                    rs = psmall.tile([P, NTOK], F32, tag="rs")
                    nc.scalar.activation(
                        out=rs[:], in_=rinv[:], func=ACT.Sqrt, bias=0.0, scale=1.0
                    )
                    nmusr = psmall.tile([P, NTOK], F32, tag="nmusr")
                    nc.vector.tensor_tensor(
                        out=nmusr[:], in0=mu[:], in1=rs[:], op=ALU.mult
                    )
                    nc.vector.tensor_scalar(
                        out=nmusr[:], in0=nmusr[:], scalar1=-1.0, scalar2=None,
                        op0=ALU.mult,
                    )
                    # ---- normalize + affine
                    xn = pmed.tile([P, NTOK * C], F32, tag="xn")
                    for q in range(NTOK):
                        qs = slice(q * C, (q + 1) * C)
                        nc.scalar.activation(
                            out=xn[:, qs],
                            in_=xres[:, qs],
                            func=ACT.Identity,
                            bias=nmusr[:, q : q + 1],
                            scale=rs[:, q : q + 1],
                        )
                    y = pmed.tile([P, NTOK * C], F32, tag="y")
                    xnr = xn[:]
                    wr = wreps[a][:]
                    yref = y[:]
                    nc.gpsimd.tensor_tensor(
                        out=_ap(yref, 0, [yref.ap[0], [C, 4], [1, C]]),
                        in0=_ap(xnr, 0, [xnr.ap[0], [C, 4], [1, C]]),
                        in1=_ap(wr, 0, [wr.ap[0], [0, 4], [1, C]]),
                        op=ALU.mult,
                    )
                    # += bias, written into the assembled output tile
                    br_ = breps[a][:]
                    yr = y[:]
                    nc.gpsimd.tensor_tensor(
                        out=_ap(xar, a * C, [xar.ap[0], [4 * C, 4], [1, C]]),
                        in0=_ap(yr, 0, [yr.ap[0], [C, 4], [1, C]]),
                        in1=_ap(br_, 0, [br_.ap[0], [0, 4], [1, C]]),
                        op=ALU.add,
                    )

                if stage >= 6:
                    # ---- store: out[2i+qh, 2j+qw, :] — 4KB contiguous rows
                    for qh in range(2):
                        for qw in range(2):
                            dst = _ap(
                                out_ap,
                                (8 * t + qh) * W * 4 * C + qw * 4 * C,
                                [[2 * W * 4 * C, 4], [2 * 4 * C, NWI], [1, 4 * C]],
                            )
                            src = _ap(
                                xar, (2 * qh + qw) * 4 * C, [xar.ap[0], [1, 4 * C]]
                            )
                            nc.sync.dma_start(out=dst, in_=src)
    return nc


def _split_multi_waits(nc):
    """Walrus on this toolchain accepts at most one embedded sync-wait per
    instruction; Tile attaches several.  Hoist all but the last wait of each
    instruction into standalone InstEventSemaphore waits on the same engine,
    inserted immediately before it (same blocking semantics)."""
    wid = 0
    for fn in nc.m.functions:
        for blk in fn.blocks:
            new_list = []
            changed = False
            for inst in blk.instructions:
                si = inst.sync_info
                if si is not None:
                    waits = list(si.on_wait)
                    if len(waits) > 1:
                        for w in waits[:-1]:
                            ev = mybir.InstEventSemaphore(
                                name=f"WSPLIT-{wid}", ins=[], outs=[]
                            )
                            wid += 1
                            ev.engine = inst.engine
                            ev.sync_info = bass_rust.SyncInfo(on_wait=[w], on_update=[])
                            new_list.append(ev)
                        inst.sync_info = bass_rust.SyncInfo(
                            on_wait=[waits[-1]], on_update=list(si.on_update)
                        )
                        changed = True
                new_list.append(inst)
            if changed:
                blk.instructions = new_list


_NC_CACHE = None


def _get_nc():
    global _NC_CACHE
    if _NC_CACHE is None:
        nc = build_kernel()
        _split_multi_waits(nc)
        _NC_CACHE = nc
    return _NC_CACHE


def kernel(**inputs) -> np.ndarray:
    nc = _get_nc()
    param_names = [f"ln{a + 1}_{s}" for a in range(4) for s in ("w", "b")]
    in_maps = []
    for ci in range(B):
        m = {
            name: np.ascontiguousarray(np.asarray(inputs[name])[ci], dtype=np.float32)
            for name in ("r", "g", "b", "ir")
        }
        for pnm in param_names:
            m[pnm] = np.ascontiguousarray(np.asarray(inputs[pnm]), dtype=np.float32)
        in_maps.append(m)
    res = run_bass_kernel_spmd(nc, in_maps, list(range(B)))
    return np.stack([res.results[ci]["out"] for ci in range(B)], axis=0)


if __name__ == "__main__":
    rng = np.random.default_rng(0)
    demo = {
        n: rng.standard_normal((B, H, W, C), dtype=np.float32)
        for n in ("r", "g", "b", "ir")
    }
    for a in range(4):
        demo[f"ln{a + 1}_w"] = rng.standard_normal(C).astype(np.float32)
        demo[f"ln{a + 1}_b"] = rng.standard_normal(C).astype(np.float32)
    o = kernel(**demo)
    print(o.shape, o.dtype)



# revision 3
# speedup vs baseline: 1.2243x; 1.2243x over previous
"""Trainium2 Bass kernel for CAttentionBlock — v5: pipelined, bf16-heavy.

Per 128-window tile, all 4 attentions batched, channels in (d,h)-permuted
order c' = d*NH + h until the Pool-side normalize un-permutes for free.

DVE-busy-minimized pipeline (drain tax ~= busy on TRN2 DVE):
  prod  = Qb*Kb                 bf16 2x   [128,16384]
  dd    = d-halve(prod)         bf16 2x in-place
  dd2   = d-halve again         bf16 2x in-place
  sred  = sum_d(4) -> s f32 1x  [128,512]
  exp (ACT), zred, recip, pn(bf16)
  pv    = pn*Kb                 bf16 2x   [128,16384]
  k01/k23/r1/xres(+qb resid)    bf16 2x add tree
  sq    = xres^2                bf16 2x TT
  sthalve(c-halve of [xres|sq]) bf16 2x
  statred -> stats f32          [128,32]
  ms/musq/vpe (DVE tiny), lnv/rs (ACT, one table set)
  t1a (Pool) un-permutes while subtracting mu; t1b *rs; t3 *w; y +b -> store

Software pipeline (1-tile stagger): iteration i issues loads(i+1) and
casts qb/kb(i+1) on ACT, phase A(i) = prod/dd/dd2 on DVE, then phase
B(i-1) = softmax..stats, so DVE never waits on ACT in steady state.
"""

import sys

for _p in ("/opt/trn_rl_repo",):
    if _p not in sys.path:
        sys.path.insert(0, _p)

import numpy as np

import bass_rust
import concourse.bass as bass
import concourse.tile as tile
from concourse import mybir
from concourse.bass_utils import run_bass_kernel_spmd

F32 = mybir.dt.float32
BF16 = mybir.dt.bfloat16
ALU = mybir.AluOpType
ACT = mybir.ActivationFunctionType
AX = mybir.AxisListType

B, H, W, C = 8, 64, 64, 256
WS = 2
NH = 8
D = C // NH            # 32
NTOK = WS * WS         # 4
NW = (H // WS) * (W // WS)
NWI = H // WS          # 32
P = 128
NTILES = NW // P       # 8
NX = 4
EPS = 1e-5
INV_SQRT_D = 1.0 / float(np.sqrt(D))

FQ = NX * NTOK * C         # 4096
FP = NX * NTOK * NTOK * C  # 16384


def _ap(ref, offset_delta, dims):
    return bass_rust.AP(ref.tensor, ref.offset + offset_delta, [list(d) for d in dims])


def build_kernel(stage=6, reps=1):
    nc = bass.Bass("TRN2", target_bir_lowering=False, debug=False)

    ins = {
        name: nc.dram_tensor(name, [H, W, C], F32, kind="ExternalInput")
        for name in ("r", "g", "b", "ir")
    }
    ln_params = []
    for a in range(4):
        wv = nc.dram_tensor(f"ln{a + 1}_w", [C], F32, kind="ExternalInput")
        bv = nc.dram_tensor(f"ln{a + 1}_b", [C], F32, kind="ExternalInput")
        ln_params.append((wv, bv))
    out = nc.dram_tensor("out", [H, W, 4 * C], F32, kind="ExternalOutput")

    in_aps = [ins[n].ap() for n in ("r", "g", "b", "ir")]
    out_ap = out.ap()
    NT = NTILES * reps

    with tile.TileContext(nc) as tc:
        with (
            tc.tile_pool(name="const", bufs=1) as pconst,
            tc.tile_pool(name="pin", bufs=1) as pin,
            tc.tile_pool(name="pk", bufs=2) as pk,
            tc.tile_pool(name="pbig", bufs=1) as pbig,
            tc.tile_pool(name="pkk", bufs=1) as pkk,
            tc.tile_pool(name="pxr", bufs=2) as pxr,
            tc.tile_pool(name="pnrm", bufs=1) as pnrm,
            tc.tile_pool(name="pout", bufs=1) as pout,
            tc.tile_pool(name="psmall", bufs=2) as psmall,
        ):
            wcat = pconst.tile([P, NX * C], F32, tag="wcat")
            bcat = pconst.tile([P, NX * C], F32, tag="bcat")
            epst = pconst.tile([P, 1], F32, tag="epst")
            nc.vector.memset(epst[:], EPS)
            for a, (wv, bv) in enumerate(ln_params):
                nc.sync.dma_start(
                    out=wcat[:, a * C : (a + 1) * C],
                    in_=_ap(wv.ap(), 0, [[0, P], [1, C]]),
                )
                nc.sync.dma_start(
                    out=bcat[:, a * C : (a + 1) * C],
                    in_=_ap(bv.ap(), 0, [[0, P], [1, C]]),
                )

            # per-tile state carried across pipeline iterations
            state = {}

            def phase_load(i):
                """loads(i) + permuting bf16 casts qb/kb(i) (ACT)."""
                t = i % NTILES
                qcat = pin.tile([P, FQ], F32, tag="qcat")
                qr = qcat[:]
                for xi in range(4):
                    for qh in range(2):
                        src = _ap(
                            in_aps[xi],
                            (8 * t + qh) * W * C,
                            [[2 * W * C, 4], [2 * C, NWI], [1, 2 * C]],
                        )
                        dst = _ap(
                            qr, xi * NTOK * C + qh * 2 * C, [qr.ap[0], [1, 2 * C]]
                        )
                        nc.sync.dma_start(out=dst, in_=src)
                qb = pk.tile([P, FQ], BF16, tag="qb")
                nc.scalar.activation(
                    out=qb[:],
                    in_=_ap(qr, 0, [qr.ap[0], [C, 16], [1, D], [D, NH]]),
                    func=ACT.Copy, bias=0.0, scale=1.0,
                )
                kb = pk.tile([P, FQ], BF16, tag="kb")
                nc.scalar.activation(
                    out=kb[:, : 3 * NTOK * C],
                    in_=qb[:, NTOK * C :],
                    func=ACT.Copy, bias=0.0, scale=1.0,
                )
                nc.scalar.activation(
                    out=kb[:, 3 * NTOK * C :],
                    in_=qb[:, NTOK * C : 2 * NTOK * C],
                    func=ACT.Copy, bias=0.0, scale=1.0,
                )
                state[i] = {"qb": qb, "kb": kb}

            def phase_a(i):
                """prod/dd/dd2/sred(i) on DVE; exp(i) on ACT."""
                st = state[i]
                qbr = st["qb"][:]
                kr = st["kb"][:]
                prod = pbig.tile([P, FP], BF16, tag="big")
                pr = prod[:]
                nc.vector.tensor_tensor(
                    out=_ap(pr, 0, [pr.ap[0], [1, FP]]),
                    in0=_ap(
                        qbr, 0, [qbr.ap[0], [NTOK * C, 4], [C, 4], [0, 4], [1, C]]
                    ),
                    in1=_ap(kr, 0, [kr.ap[0], [NTOK * C, 4], [0, 4], [C, 4], [1, C]]),
                    op=ALU.mult,
                )
                # two in-place d-halvings: c' = d*8+h so d-halves are
                # contiguous 128- then 64-elem runs inside each 256 block
                half_dims = [pr.ap[0], [1024, 16], [256, 4], [1, 128]]
                nc.vector.tensor_tensor(
                    out=_ap(pr, 0, half_dims),
                    in0=_ap(pr, 0, half_dims),
                    in1=_ap(pr, 128, half_dims),
                    op=ALU.add,
                )
                q_dims = [pr.ap[0], [1024, 16], [256, 4], [1, 64]]
                nc.vector.tensor_tensor(
                    out=_ap(pr, 0, q_dims),
                    in0=_ap(pr, 0, q_dims),
                    in1=_ap(pr, 64, q_dims),
                    op=ALU.add,
                )
                if stage < 2:
                    st["prod"] = prod
                    return
                s = psmall.tile([P, 512], F32, tag="s", bufs=2)
                sr = s[:]
                nc.vector.reduce_sum(
                    out=_ap(sr, 0, [sr.ap[0], [32, 16], [8, 4], [1, 8]]),
                    in_=_ap(pr, 0, [pr.ap[0], [1024, 16], [256, 4], [1, 8], [8, 8]]),
                    axis=AX.X,
                )
                # exp in place (same tile, cross-iteration lifetime via bufs=2)
                nc.scalar.activation(
                    out=s[:], in_=s[:], func=ACT.Exp, bias=0.0, scale=INV_SQRT_D
                )
                st["prod"] = prod
                st["e"] = s

            def phase_b(i):
                """softmax tail, AV, residual tree, stats (DVE); LN tail."""
                st = state[i]
                if stage < 3:
                    return
                e = st["e"]
                er = e[:]
                kr = st["kb"][:]
                qbr = st["qb"][:]
                z = psmall.tile([P, 128], F32, tag="z", bufs=1)
                nc.vector.reduce_sum(
                    out=z[:],
                    in_=_ap(er, 0, [er.ap[0], [32, 16], [1, 8], [8, 4]]),
                    axis=AX.X,
                )
                rz = psmall.tile([P, 128], F32, tag="rz", bufs=1)
                nc.vector.reciprocal(out=rz[:], in_=z[:])
                pn = psmall.tile([P, 512], BF16, tag="pn", bufs=1)
                rzr = rz[:]
                nc.vector.tensor_tensor(
                    out=pn[:],
                    in0=e[:],
                    in1=_ap(rzr, 0, [rzr.ap[0], [8, 16], [0, 4], [1, 8]]),
                    op=ALU.mult,
                )
                if stage < 4:
                    return
                pnr = pn[:]
                pv = pbig.tile([P, FP], BF16, tag="big")
                pvr = pv[:]
                nc.vector.tensor_tensor(
                    out=_ap(pvr, 0, [pvr.ap[0], [1, FP]]),
                    in0=_ap(pnr, 0, [pnr.ap[0], [32, 16], [8, 4], [0, D], [1, 8]]),
                    in1=_ap(kr, 0, [kr.ap[0], [NTOK * C, 4], [0, 4], [C, 4], [1, C]]),
                    op=ALU.mult,
                )
                # k-sum tree + residual, all bf16 2x, output stays permuted
                kk = pkk.tile([P, 2 * FQ], BF16, tag="kk")
                kkr = kk[:]
                qk_dims = [pvr.ap[0], [1024, 16], [1, 256]]
                nc.vector.tensor_tensor(
                    out=_ap(kkr, 0, [kkr.ap[0], [1, FQ]]),
                    in0=_ap(pvr, 0 * C, qk_dims),
                    in1=_ap(pvr, 1 * C, qk_dims),
                    op=ALU.add,
                )
                nc.vector.tensor_tensor(
                    out=_ap(kkr, FQ, [kkr.ap[0], [1, FQ]]),
                    in0=_ap(pvr, 2 * C, qk_dims),
                    in1=_ap(pvr, 3 * C, qk_dims),
                    op=ALU.add,
                )
                xrsq = pxr.tile([P, 2 * FQ], BF16, tag="xrsq")
                xr = xrsq[:]
                nc.vector.tensor_tensor(
                    out=_ap(xr, 0, [xr.ap[0], [1, FQ]]),
                    in0=_ap(kkr, 0, [kkr.ap[0], [1, FQ]]),
                    in1=_ap(kkr, FQ, [kkr.ap[0], [1, FQ]]),
                    op=ALU.add,
                )
                nc.vector.tensor_tensor(
                    out=_ap(xr, 0, [xr.ap[0], [1, FQ]]),
                    in0=_ap(xr, 0, [xr.ap[0], [1, FQ]]),
                    in1=_ap(qbr, 0, [qbr.ap[0], [1, FQ]]),
                    op=ALU.add,
                )
                st["xrsq"] = xrsq
                if stage < 5:
                    return
                # stats: sq, combined c-halve of [xres|sq], reduce
                nc.vector.tensor_tensor(
                    out=_ap(xr, FQ, [xr.ap[0], [1, FQ]]),
                    in0=_ap(xr, 0, [xr.ap[0], [1, FQ]]),
                    in1=_ap(xr, 0, [xr.ap[0], [1, FQ]]),
                    op=ALU.mult,
                )
                sth = pkk.tile([P, FQ], BF16, tag="kk")
                str0 = sth[:]
                h_dims = [xr.ap[0], [C, 32], [1, 128]]
                nc.vector.tensor_tensor(
                    out=_ap(str0, 0, [str0.ap[0], [1, FQ]]),
                    in0=_ap(xr, 0, h_dims),
                    in1=_ap(xr, 128, h_dims),
                    op=ALU.add,
                )
                stats = psmall.tile([P, 32], F32, tag="stats", bufs=1)
                str_ = stats[:]
                nc.vector.reduce_sum(
                    out=_ap(str_, 0, [str_.ap[0], [1, 32]]),
                    in_=_ap(str0, 0, [str0.ap[0], [128, 32], [1, 128]]),
                    axis=AX.X,
                )
                ms = psmall.tile([P, 32], F32, tag="ms", bufs=2)
                nc.vector.tensor_scalar(
                    out=ms[:], in0=stats[:], scalar1=1.0 / C, scalar2=None,
                    op0=ALU.mult,
                )
                musq = psmall.tile([P, 16], F32, tag="musq", bufs=1)
                nc.vector.tensor_tensor(
                    out=musq[:], in0=ms[:, :16], in1=ms[:, :16], op=ALU.mult
                )
                vpe = psmall.tile([P, 16], F32, tag="vpe", bufs=1)
                nc.vector.tensor_tensor(
                    out=vpe[:], in0=ms[:, 16:], in1=musq[:], op=ALU.subtract
                )
                lnv = psmall.tile([P, 16], F32, tag="lnv", bufs=1)
                nc.scalar.activation(
                    out=lnv[:], in_=vpe[:], func=ACT.Ln, bias=epst[:], scale=1.0
                )
                rs = psmall.tile([P, 16], F32, tag="rs", bufs=2)
                nc.scalar.activation(
                    out=rs[:], in_=lnv[:], func=ACT.Exp, bias=0.0, scale=-0.5
                )
                # normalize on Pool; t1a un-permutes (d,h)->true c (3 free dims)
                t1 = pnrm.tile([P, FQ], F32, tag="t1")
                t1r = t1[:]
                msr = ms[:]
                nc.gpsimd.tensor_tensor(
                    out=_ap(t1r, 0, [t1r.ap[0], [C, 16], [1, D], [D, NH]]),
                    in0=_ap(xr, 0, [xr.ap[0], [1, FQ]]),
                    in1=_ap(msr, 0, [msr.ap[0], [1, 16], [0, C]]),
                    op=ALU.subtract,
                )
                rsr = rs[:]
                nc.gpsimd.tensor_tensor(
                    out=t1r,
                    in0=t1r,
                    in1=_ap(rsr, 0, [rsr.ap[0], [1, 16], [0, C]]),
                    op=ALU.mult,
                )
                t3 = pnrm.tile([P, FQ], F32, tag="t3")
                wr = wcat[:]
                nc.gpsimd.tensor_tensor(
                    out=t3[:],
                    in0=t1r,
                    in1=_ap(wr, 0, [wr.ap[0], [C, 4], [0, 4], [1, C]]),
                    op=ALU.mult,
                )
                xout = pout.tile([P, NTOK * NX * C], F32, tag="xout")
                xor_ = xout[:]
                br = bcat[:]
                nc.gpsimd.tensor_tensor(
                    out=_ap(xor_, 0, [xor_.ap[0], [C, 4], [NX * C, 4], [1, C]]),
                    in0=t3[:],
                    in1=_ap(br, 0, [br.ap[0], [C, 4], [0, 4], [1, C]]),
                    op=ALU.add,
                )
                if stage >= 6:
                    t = i % NTILES
                    for qh in range(2):
                        dst = _ap(
                            out_ap,
                            (8 * t + qh) * W * NX * C,
                            [[2 * W * NX * C, 4], [2 * NX * C, NWI], [1, 2 * NX * C]],
                        )
                        src = _ap(
                            xor_, qh * 2 * NX * C, [xor_.ap[0], [1, 2 * NX * C]]
                        )
                        nc.sync.dma_start(out=dst, in_=src)
                del state[i]

            # ---- pipelined issue: B(i-1) first (big bufs=1), loads one ahead
            phase_load(0)
            for i in range(NT):
                if i >= 1:
                    phase_b(i - 1)
                if i + 1 < NT:
                    phase_load(i + 1)
                phase_a(i)
            phase_b(NT - 1)
    return nc


def _split_multi_waits(nc):
    wid = 0
    for fn in nc.m.functions:
        for blk in fn.blocks:
            new_list = []
            changed = False
            for inst in blk.instructions:
                si = inst.sync_info
                if si is not None:
                    waits = list(si.on_wait)
                    if len(waits) > 1:
                        for w in waits[:-1]:
                            ev = mybir.InstEventSemaphore(
                                name=f"WSPLIT-{wid}", ins=[], outs=[]
                            )
                            wid += 1
                            ev.engine = inst.engine
                            ev.sync_info = bass_rust.SyncInfo(on_wait=[w], on_update=[])
                            new_list.append(ev)
                        inst.sync_info = bass_rust.SyncInfo(
                            on_wait=[waits[-1]], on_update=list(si.on_update)
                        )
                        changed = True
                new_list.append(inst)
            if changed:
                blk.instructions = new_list


_NC_CACHE = None


def _get_nc():
    global _NC_CACHE
    if _NC_CACHE is None:
        nc = build_kernel()
        _split_multi_waits(nc)
        _NC_CACHE = nc
    return _NC_CACHE


def kernel(**inputs) -> np.ndarray:
    nc = _get_nc()
    param_names = [f"ln{a + 1}_{s}" for a in range(4) for s in ("w", "b")]
    in_maps = []
    for ci in range(B):
        m = {
            name: np.ascontiguousarray(np.asarray(inputs[name])[ci], dtype=np.float32)
            for name in ("r", "g", "b", "ir")
        }
        for pnm in param_names:
            m[pnm] = np.ascontiguousarray(np.asarray(inputs[pnm]), dtype=np.float32)
        in_maps.append(m)
    res = run_bass_kernel_spmd(nc, in_maps, list(range(B)))
    return np.stack([res.results[ci]["out"] for ci in range(B)], axis=0)


if __name__ == "__main__":
    rng = np.random.default_rng(0)
    demo = {
        n: rng.standard_normal((B, H, W, C), dtype=np.float32)
        for n in ("r", "g", "b", "ir")
    }
    for a in range(4):
        demo[f"ln{a + 1}_w"] = rng.standard_normal(C).astype(np.float32)
        demo[f"ln{a + 1}_b"] = rng.standard_normal(C).astype(np.float32)
    o = kernel(**demo)
    print(o.shape, o.dtype)


# revision 4
# speedup vs baseline: 1.2585x; 1.0279x over previous
"""Trainium2 Bass kernel for CAttentionBlock — v5: pipelined, bf16-heavy.

Per 128-window tile, all 4 attentions batched, channels in (d,h)-permuted
order c' = d*NH + h until the Pool-side normalize un-permutes for free.

DVE-busy-minimized pipeline (drain tax ~= busy on TRN2 DVE):
  prod  = Qb*Kb                 bf16 2x   [128,16384]
  dd    = d-halve(prod)         bf16 2x in-place
  dd2   = d-halve again         bf16 2x in-place
  sred  = sum_d(4) -> s f32 1x  [128,512]
  exp (ACT), zred, recip, pn(bf16)
  pv    = pn*Kb                 bf16 2x   [128,16384]
  k01/k23/r1/xres(+qb resid)    bf16 2x add tree
  sq    = xres^2                bf16 2x TT
  sthalve(c-halve of [xres|sq]) bf16 2x
  statred -> stats f32          [128,32]
  ms/musq/vpe (DVE tiny), lnv/rs (ACT, one table set)
  t1a (Pool) un-permutes while subtracting mu; t1b *rs; t3 *w; y +b -> store

Software pipeline (1-tile stagger): iteration i issues loads(i+1) and
casts qb/kb(i+1) on ACT, phase A(i) = prod/dd/dd2 on DVE, then phase
B(i-1) = softmax..stats, so DVE never waits on ACT in steady state.
"""

import sys

for _p in ("/opt/trn_rl_repo",):
    if _p not in sys.path:
        sys.path.insert(0, _p)

import numpy as np

import bass_rust
import concourse.bass as bass
import concourse.tile as tile
from concourse import mybir
from concourse.bass_utils import run_bass_kernel_spmd

F32 = mybir.dt.float32
BF16 = mybir.dt.bfloat16
ALU = mybir.AluOpType
ACT = mybir.ActivationFunctionType
AX = mybir.AxisListType

B, H, W, C = 8, 64, 64, 256
WS = 2
NH = 8
D = C // NH            # 32
NTOK = WS * WS         # 4
NW = (H // WS) * (W // WS)
NWI = H // WS          # 32
P = 128
NTILES = NW // P       # 8
NX = 4
EPS = 1e-5
INV_SQRT_D = 1.0 / float(np.sqrt(D))

FQ = NX * NTOK * C         # 4096
FP = NX * NTOK * NTOK * C  # 16384


def _ap(ref, offset_delta, dims):
    return bass_rust.AP(ref.tensor, ref.offset + offset_delta, [list(d) for d in dims])


def build_kernel(stage=6, reps=1):
    nc = bass.Bass("TRN2", target_bir_lowering=False, debug=False)

    ins = {
        name: nc.dram_tensor(name, [H, W, C], F32, kind="ExternalInput")
        for name in ("r", "g", "b", "ir")
    }
    ln_params = []
    for a in range(4):
        wv = nc.dram_tensor(f"ln{a + 1}_w", [C], F32, kind="ExternalInput")
        bv = nc.dram_tensor(f"ln{a + 1}_b", [C], F32, kind="ExternalInput")
        ln_params.append((wv, bv))
    out = nc.dram_tensor("out", [H, W, 4 * C], F32, kind="ExternalOutput")

    in_aps = [ins[n].ap() for n in ("r", "g", "b", "ir")]
    out_ap = out.ap()
    NT = NTILES * reps

    with tile.TileContext(nc) as tc:
        with (
            tc.tile_pool(name="const", bufs=1) as pconst,
            tc.tile_pool(name="pin", bufs=1) as pin,
            tc.tile_pool(name="pk", bufs=2) as pk,
            tc.tile_pool(name="pbig", bufs=1) as pbig,
            tc.tile_pool(name="pkk", bufs=1) as pkk,
            tc.tile_pool(name="pxr", bufs=2) as pxr,
            tc.tile_pool(name="pnrm", bufs=1) as pnrm,
            tc.tile_pool(name="pout", bufs=1) as pout,
            tc.tile_pool(name="psmall", bufs=2) as psmall,
        ):
            wcat = pconst.tile([P, NX * C], F32, tag="wcat")
            bcat = pconst.tile([P, NX * C], F32, tag="bcat")
            epst = pconst.tile([P, 1], F32, tag="epst")
            nc.vector.memset(epst[:], EPS)
            for a, (wv, bv) in enumerate(ln_params):
                nc.sync.dma_start(
                    out=wcat[:, a * C : (a + 1) * C],
                    in_=_ap(wv.ap(), 0, [[0, P], [1, C]]),
                )
                nc.sync.dma_start(
                    out=bcat[:, a * C : (a + 1) * C],
                    in_=_ap(bv.ap(), 0, [[0, P], [1, C]]),
                )

            # per-tile state carried across pipeline iterations
            state = {}

            def phase_load(i):
                """loads(i) + permuting bf16 casts qb/kb(i) (ACT)."""
                t = i % NTILES
                qcat = pin.tile([P, FQ], F32, tag="qcat")
                qr = qcat[:]
                for xi in range(4):
                    for qh in range(2):
                        src = _ap(
                            in_aps[xi],
                            (8 * t + qh) * W * C,
                            [[2 * W * C, 4], [2 * C, NWI], [1, 2 * C]],
                        )
                        dst = _ap(
                            qr, xi * NTOK * C + qh * 2 * C, [qr.ap[0], [1, 2 * C]]
                        )
                        nc.sync.dma_start(out=dst, in_=src)
                qb = pk.tile([P, FQ], BF16, tag="qb")
                nc.scalar.activation(
                    out=qb[:],
                    in_=_ap(qr, 0, [qr.ap[0], [C, 16], [1, D], [D, NH]]),
                    func=ACT.Copy, bias=0.0, scale=1.0,
                )
                kb = pk.tile([P, FQ], BF16, tag="kb")
                nc.scalar.activation(
                    out=kb[:, : 3 * NTOK * C],
                    in_=qb[:, NTOK * C :],
                    func=ACT.Copy, bias=0.0, scale=1.0,
                )
                nc.scalar.activation(
                    out=kb[:, 3 * NTOK * C :],
                    in_=qb[:, NTOK * C : 2 * NTOK * C],
                    func=ACT.Copy, bias=0.0, scale=1.0,
                )
                state[i] = {"qb": qb, "kb": kb}

            def phase_a(i):
                """prod/dd/dd2/sred(i) on DVE; exp(i) on ACT."""
                st = state[i]
                qbr = st["qb"][:]
                kr = st["kb"][:]
                prod = pbig.tile([P, FP], BF16, tag="big")
                pr = prod[:]
                nc.vector.tensor_tensor(
                    out=_ap(pr, 0, [pr.ap[0], [1, FP]]),
                    in0=_ap(
                        qbr, 0, [qbr.ap[0], [NTOK * C, 4], [C, 4], [0, 4], [1, C]]
                    ),
                    in1=_ap(kr, 0, [kr.ap[0], [NTOK * C, 4], [0, 4], [C, 4], [1, C]]),
                    op=ALU.mult,
                )
                # two in-place d-halvings: c' = d*8+h so d-halves are
                # contiguous 128- then 64-elem runs inside each 256 block
                half_dims = [pr.ap[0], [1024, 16], [256, 4], [1, 128]]
                nc.vector.tensor_tensor(
                    out=_ap(pr, 0, half_dims),
                    in0=_ap(pr, 0, half_dims),
                    in1=_ap(pr, 128, half_dims),
                    op=ALU.add,
                )
                q_dims = [pr.ap[0], [1024, 16], [256, 4], [1, 64]]
                nc.vector.tensor_tensor(
                    out=_ap(pr, 0, q_dims),
                    in0=_ap(pr, 0, q_dims),
                    in1=_ap(pr, 64, q_dims),
                    op=ALU.add,
                )
                o_dims = [pr.ap[0], [1024, 16], [256, 4], [1, 32]]
                nc.vector.tensor_tensor(
                    out=_ap(pr, 0, o_dims),
                    in0=_ap(pr, 0, o_dims),
                    in1=_ap(pr, 32, o_dims),
                    op=ALU.add,
                )
                if stage < 2:
                    st["prod"] = prod
                    return
                s = psmall.tile([P, 512], F32, tag="s", bufs=2)
                sr = s[:]
                nc.vector.reduce_sum(
                    out=_ap(sr, 0, [sr.ap[0], [32, 16], [8, 4], [1, 8]]),
                    in_=_ap(pr, 0, [pr.ap[0], [1024, 16], [256, 4], [1, 8], [8, 4]]),
                    axis=AX.X,
                )
                # exp in place (same tile, cross-iteration lifetime via bufs=2)
                nc.scalar.activation(
                    out=s[:], in_=s[:], func=ACT.Exp, bias=0.0, scale=INV_SQRT_D
                )
                st["prod"] = prod
                st["e"] = s

            def phase_b(i):
                """softmax tail, AV, residual tree, stats (DVE); LN tail."""
                st = state[i]
                if stage < 3:
                    return
                e = st["e"]
                er = e[:]
                kr = st["kb"][:]
                qbr = st["qb"][:]
                z = psmall.tile([P, 128], F32, tag="z", bufs=1)
                nc.vector.reduce_sum(
                    out=z[:],
                    in_=_ap(er, 0, [er.ap[0], [32, 16], [1, 8], [8, 4]]),
                    axis=AX.X,
                )
                rz = psmall.tile([P, 128], F32, tag="rz", bufs=1)
                nc.vector.reciprocal(out=rz[:], in_=z[:])
                pn = psmall.tile([P, 512], BF16, tag="pn", bufs=1)
                rzr = rz[:]
                nc.vector.tensor_tensor(
                    out=pn[:],
                    in0=e[:],
                    in1=_ap(rzr, 0, [rzr.ap[0], [8, 16], [0, 4], [1, 8]]),
                    op=ALU.mult,
                )
                if stage < 4:
                    return
                pnr = pn[:]
                pv = pbig.tile([P, FP], BF16, tag="big")
                pvr = pv[:]
                nc.vector.tensor_tensor(
                    out=_ap(pvr, 0, [pvr.ap[0], [1, FP]]),
                    in0=_ap(pnr, 0, [pnr.ap[0], [32, 16], [8, 4], [0, D], [1, 8]]),
                    in1=_ap(kr, 0, [kr.ap[0], [NTOK * C, 4], [0, 4], [C, 4], [1, C]]),
                    op=ALU.mult,
                )
                # k-sum tree + residual, all bf16 2x, output stays permuted
                kk = pkk.tile([P, 2 * FQ], BF16, tag="kk")
                kkr = kk[:]
                qk_dims = [pvr.ap[0], [1024, 16], [1, 256]]
                nc.vector.tensor_tensor(
                    out=_ap(kkr, 0, [kkr.ap[0], [1, FQ]]),
                    in0=_ap(pvr, 0 * C, qk_dims),
                    in1=_ap(pvr, 1 * C, qk_dims),
                    op=ALU.add,
                )
                nc.vector.tensor_tensor(
                    out=_ap(kkr, FQ, [kkr.ap[0], [1, FQ]]),
                    in0=_ap(pvr, 2 * C, qk_dims),
                    in1=_ap(pvr, 3 * C, qk_dims),
                    op=ALU.add,
                )
                xrsq = pxr.tile([P, 2 * FQ], BF16, tag="xrsq")
                xr = xrsq[:]
                nc.vector.tensor_tensor(
                    out=_ap(xr, 0, [xr.ap[0], [1, FQ]]),
                    in0=_ap(kkr, 0, [kkr.ap[0], [1, FQ]]),
                    in1=_ap(kkr, FQ, [kkr.ap[0], [1, FQ]]),
                    op=ALU.add,
                )
                nc.vector.tensor_tensor(
                    out=_ap(xr, 0, [xr.ap[0], [1, FQ]]),
                    in0=_ap(xr, 0, [xr.ap[0], [1, FQ]]),
                    in1=_ap(qbr, 0, [qbr.ap[0], [1, FQ]]),
                    op=ALU.add,
                )
                st["xrsq"] = xrsq
                if stage < 5:
                    return
                # stats: sq, combined c-halve of [xres|sq], reduce
                nc.vector.tensor_tensor(
                    out=_ap(xr, FQ, [xr.ap[0], [1, FQ]]),
                    in0=_ap(xr, 0, [xr.ap[0], [1, FQ]]),
                    in1=_ap(xr, 0, [xr.ap[0], [1, FQ]]),
                    op=ALU.mult,
                )
                sth = pkk.tile([P, FQ], BF16, tag="kk")
                str0 = sth[:]
                h_dims = [xr.ap[0], [C, 32], [1, 128]]
                nc.vector.tensor_tensor(
                    out=_ap(str0, 0, [str0.ap[0], [1, FQ]]),
                    in0=_ap(xr, 0, h_dims),
                    in1=_ap(xr, 128, h_dims),
                    op=ALU.add,
                )
                h2_dims = [str0.ap[0], [128, 32], [1, 64]]
                nc.vector.tensor_tensor(
                    out=_ap(str0, 0, h2_dims),
                    in0=_ap(str0, 0, h2_dims),
                    in1=_ap(str0, 64, h2_dims),
                    op=ALU.add,
                )
                stats = psmall.tile([P, 32], F32, tag="stats", bufs=1)
                str_ = stats[:]
                nc.vector.reduce_sum(
                    out=_ap(str_, 0, [str_.ap[0], [1, 32]]),
                    in_=_ap(str0, 0, [str0.ap[0], [128, 32], [1, 64]]),
                    axis=AX.X,
                )
                ms = psmall.tile([P, 32], F32, tag="ms", bufs=2)
                nc.vector.tensor_scalar(
                    out=ms[:], in0=stats[:], scalar1=1.0 / C, scalar2=None,
                    op0=ALU.mult,
                )
                musq = psmall.tile([P, 16], F32, tag="musq", bufs=1)
                nc.vector.tensor_tensor(
                    out=musq[:], in0=ms[:, :16], in1=ms[:, :16], op=ALU.mult
                )
                vpe = psmall.tile([P, 16], F32, tag="vpe", bufs=1)
                nc.vector.tensor_tensor(
                    out=vpe[:], in0=ms[:, 16:], in1=musq[:], op=ALU.subtract
                )
                lnv = psmall.tile([P, 16], F32, tag="lnv", bufs=1)
                nc.scalar.activation(
                    out=lnv[:], in_=vpe[:], func=ACT.Ln, bias=epst[:], scale=1.0
                )
                rs = psmall.tile([P, 16], F32, tag="rs", bufs=2)
                nc.scalar.activation(
                    out=rs[:], in_=lnv[:], func=ACT.Exp, bias=0.0, scale=-0.5
                )
                # normalize on Pool; t1a un-permutes (d,h)->true c (3 free dims)
                t1 = pnrm.tile([P, FQ], F32, tag="t1")
                t1r = t1[:]
                msr = ms[:]
                nc.gpsimd.tensor_tensor(
                    out=_ap(t1r, 0, [t1r.ap[0], [C, 16], [1, D], [D, NH]]),
                    in0=_ap(xr, 0, [xr.ap[0], [1, FQ]]),
                    in1=_ap(msr, 0, [msr.ap[0], [1, 16], [0, C]]),
                    op=ALU.subtract,
                )
                rsr = rs[:]
                nc.gpsimd.tensor_tensor(
                    out=t1r,
                    in0=t1r,
                    in1=_ap(rsr, 0, [rsr.ap[0], [1, 16], [0, C]]),
                    op=ALU.mult,
                )
                t3 = pnrm.tile([P, FQ], F32, tag="t3")
                wr = wcat[:]
                nc.gpsimd.tensor_tensor(
                    out=t3[:],
                    in0=t1r,
                    in1=_ap(wr, 0, [wr.ap[0], [C, 4], [0, 4], [1, C]]),
                    op=ALU.mult,
                )
                xout = pout.tile([P, NTOK * NX * C], F32, tag="xout")
                xor_ = xout[:]
                br = bcat[:]
                nc.gpsimd.tensor_tensor(
                    out=_ap(xor_, 0, [xor_.ap[0], [C, 4], [NX * C, 4], [1, C]]),
                    in0=t3[:],
                    in1=_ap(br, 0, [br.ap[0], [C, 4], [0, 4], [1, C]]),
                    op=ALU.add,
                )
                if stage >= 6:
                    t = i % NTILES
                    for qh in range(2):
                        dst = _ap(
                            out_ap,
                            (8 * t + qh) * W * NX * C,
                            [[2 * W * NX * C, 4], [2 * NX * C, NWI], [1, 2 * NX * C]],
                        )
                        src = _ap(
                            xor_, qh * 2 * NX * C, [xor_.ap[0], [1, 2 * NX * C]]
                        )
                        nc.sync.dma_start(out=dst, in_=src)
                del state[i]

            # ---- pipelined issue: B(i-1) first (big bufs=1), loads one ahead
            phase_load(0)
            for i in range(NT):
                if i >= 1:
                    phase_b(i - 1)
                if i + 1 < NT:
                    phase_load(i + 1)
                phase_a(i)
            phase_b(NT - 1)
    return nc


def _split_multi_waits(nc):
    wid = 0
    for fn in nc.m.functions:
        for blk in fn.blocks:
            new_list = []
            changed = False
            for inst in blk.instructions:
                si = inst.sync_info
                if si is not None:
                    waits = list(si.on_wait)
                    if len(waits) > 1:
                        for w in waits[:-1]:
                            ev = mybir.InstEventSemaphore(
                                name=f"WSPLIT-{wid}", ins=[], outs=[]
                            )
                            wid += 1
                            ev.engine = inst.engine
                            ev.sync_info = bass_rust.SyncInfo(on_wait=[w], on_update=[])
                            new_list.append(ev)
                        inst.sync_info = bass_rust.SyncInfo(
                            on_wait=[waits[-1]], on_update=list(si.on_update)
                        )
                        changed = True
                new_list.append(inst)
            if changed:
                blk.instructions = new_list


_NC_CACHE = None


def _get_nc():
    global _NC_CACHE
    if _NC_CACHE is None:
        nc = build_kernel()
        _split_multi_waits(nc)
        _NC_CACHE = nc
    return _NC_CACHE


def kernel(**inputs) -> np.ndarray:
    nc = _get_nc()
    param_names = [f"ln{a + 1}_{s}" for a in range(4) for s in ("w", "b")]
    in_maps = []
    for ci in range(B):
        m = {
            name: np.ascontiguousarray(np.asarray(inputs[name])[ci], dtype=np.float32)
            for name in ("r", "g", "b", "ir")
        }
        for pnm in param_names:
            m[pnm] = np.ascontiguousarray(np.asarray(inputs[pnm]), dtype=np.float32)
        in_maps.append(m)
    res = run_bass_kernel_spmd(nc, in_maps, list(range(B)))
    return np.stack([res.results[ci]["out"] for ci in range(B)], axis=0)


if __name__ == "__main__":
    rng = np.random.default_rng(0)
    demo = {
        n: rng.standard_normal((B, H, W, C), dtype=np.float32)
        for n in ("r", "g", "b", "ir")
    }
    for a in range(4):
        demo[f"ln{a + 1}_w"] = rng.standard_normal(C).astype(np.float32)
        demo[f"ln{a + 1}_b"] = rng.standard_normal(C).astype(np.float32)
    o = kernel(**demo)
    print(o.shape, o.dtype)


# revision 5
# speedup vs baseline: 1.3194x; 1.0484x over previous
"""Trainium2 Bass kernel for CAttentionBlock — v5: pipelined, bf16-heavy.

Per 128-window tile, all 4 attentions batched, channels in (d,h)-permuted
order c' = d*NH + h until the Pool-side normalize un-permutes for free.

DVE-busy-minimized pipeline (drain tax ~= busy on TRN2 DVE):
  prod  = Qb*Kb                 bf16 2x   [128,16384]
  dd    = d-halve(prod)         bf16 2x in-place
  dd2   = d-halve again         bf16 2x in-place
  sred  = sum_d(4) -> s f32 1x  [128,512]
  exp (ACT), zred, recip, pn(bf16)
  pv    = pn*Kb                 bf16 2x   [128,16384]
  k01/k23/r1/xres(+qb resid)    bf16 2x add tree
  sq    = xres^2                bf16 2x TT
  sthalve(c-halve of [xres|sq]) bf16 2x
  statred -> stats f32          [128,32]
  ms/musq/vpe (DVE tiny), lnv/rs (ACT, one table set)
  t1a (Pool) un-permutes while subtracting mu; t1b *rs; t3 *w; y +b -> store

Software pipeline (1-tile stagger): iteration i issues loads(i+1) and
casts qb/kb(i+1) on ACT, phase A(i) = prod/dd/dd2 on DVE, then phase
B(i-1) = softmax..stats, so DVE never waits on ACT in steady state.
"""

import sys

for _p in ("/opt/trn_rl_repo",):
    if _p not in sys.path:
        sys.path.insert(0, _p)

import numpy as np

import bass_rust
import concourse.bass as bass
import concourse.tile as tile
from concourse import mybir
from concourse.bass_utils import run_bass_kernel_spmd

F32 = mybir.dt.float32
BF16 = mybir.dt.bfloat16
ALU = mybir.AluOpType
ACT = mybir.ActivationFunctionType
AX = mybir.AxisListType

B, H, W, C = 8, 64, 64, 256
WS = 2
NH = 8
D = C // NH            # 32
NTOK = WS * WS         # 4
NW = (H // WS) * (W // WS)
NWI = H // WS          # 32
P = 128
NTILES = NW // P       # 8
NX = 4
EPS = 1e-5
INV_SQRT_D = 1.0 / float(np.sqrt(D))

FQ = NX * NTOK * C         # 4096
FP = NX * NTOK * NTOK * C  # 16384


def _ap(ref, offset_delta, dims):
    return bass_rust.AP(ref.tensor, ref.offset + offset_delta, [list(d) for d in dims])


def build_kernel(stage=6, reps=1):
    nc = bass.Bass("TRN2", target_bir_lowering=False, debug=False)

    ins = {
        name: nc.dram_tensor(name, [H, W, C], F32, kind="ExternalInput")
        for name in ("r", "g", "b", "ir")
    }
    ln_params = []
    for a in range(4):
        wv = nc.dram_tensor(f"ln{a + 1}_w", [C], F32, kind="ExternalInput")
        bv = nc.dram_tensor(f"ln{a + 1}_b", [C], F32, kind="ExternalInput")
        ln_params.append((wv, bv))
    out = nc.dram_tensor("out", [H, W, 4 * C], F32, kind="ExternalOutput")

    in_aps = [ins[n].ap() for n in ("r", "g", "b", "ir")]
    out_ap = out.ap()
    NT = NTILES * reps

    with tile.TileContext(nc) as tc:
        with (
            tc.tile_pool(name="const", bufs=1) as pconst,
            tc.tile_pool(name="pin", bufs=1) as pin,
            tc.tile_pool(name="pk", bufs=2) as pk,
            tc.tile_pool(name="pbig", bufs=1) as pbig,
            tc.tile_pool(name="pkk", bufs=1) as pkk,
            tc.tile_pool(name="pxr", bufs=2) as pxr,
            tc.tile_pool(name="pnrm", bufs=1) as pnrm,
            tc.tile_pool(name="pout", bufs=1) as pout,
            tc.tile_pool(name="psmall", bufs=2) as psmall,
        ):
            wcat = pconst.tile([P, NX * C], F32, tag="wcat")
            bcat = pconst.tile([P, NX * C], F32, tag="bcat")
            epst = pconst.tile([P, 1], F32, tag="epst")
            nc.vector.memset(epst[:], EPS)
            for a, (wv, bv) in enumerate(ln_params):
                nc.sync.dma_start(
                    out=wcat[:, a * C : (a + 1) * C],
                    in_=_ap(wv.ap(), 0, [[0, P], [1, C]]),
                )
                nc.sync.dma_start(
                    out=bcat[:, a * C : (a + 1) * C],
                    in_=_ap(bv.ap(), 0, [[0, P], [1, C]]),
                )

            # per-tile state carried across pipeline iterations
            state = {}

            def phase_load(i):
                """loads(i) + permuting bf16 casts qb/kb(i) (ACT)."""
                t = i % NTILES
                qcat = pin.tile([P, FQ], F32, tag="qcat")
                qr = qcat[:]
                for xi in range(4):
                    for qh in range(2):
                        src = _ap(
                            in_aps[xi],
                            (8 * t + qh) * W * C,
                            [[2 * W * C, 4], [2 * C, NWI], [1, 2 * C]],
                        )
                        dst = _ap(
                            qr, xi * NTOK * C + qh * 2 * C, [qr.ap[0], [1, 2 * C]]
                        )
                        nc.sync.dma_start(out=dst, in_=src)
                qb = pk.tile([P, FQ], BF16, tag="qb")
                nc.scalar.activation(
                    out=qb[:],
                    in_=_ap(qr, 0, [qr.ap[0], [C, 16], [1, D], [D, NH]]),
                    func=ACT.Copy, bias=0.0, scale=1.0,
                )
                kb = pk.tile([P, FQ], BF16, tag="kb")
                nc.scalar.activation(
                    out=kb[:, : 3 * NTOK * C],
                    in_=qb[:, NTOK * C :],
                    func=ACT.Copy, bias=0.0, scale=1.0,
                )
                nc.scalar.activation(
                    out=kb[:, 3 * NTOK * C :],
                    in_=qb[:, NTOK * C : 2 * NTOK * C],
                    func=ACT.Copy, bias=0.0, scale=1.0,
                )
                state[i] = {"qb": qb, "kb": kb}

            def phase_a(i):
                """prod/dd/dd2/sred(i) on DVE; exp(i) on ACT."""
                st = state[i]
                qbr = st["qb"][:]
                kr = st["kb"][:]
                prod = pbig.tile([P, FP], BF16, tag="big")
                pr = prod[:]
                nc.vector.tensor_tensor(
                    out=_ap(pr, 0, [pr.ap[0], [1, FP]]),
                    in0=_ap(
                        qbr, 0, [qbr.ap[0], [NTOK * C, 4], [C, 4], [0, 4], [1, C]]
                    ),
                    in1=_ap(kr, 0, [kr.ap[0], [NTOK * C, 4], [0, 4], [C, 4], [1, C]]),
                    op=ALU.mult,
                )
                # two in-place d-halvings: c' = d*8+h so d-halves are
                # contiguous 128- then 64-elem runs inside each 256 block
                half_dims = [pr.ap[0], [1024, 16], [256, 4], [1, 128]]
                nc.vector.tensor_tensor(
                    out=_ap(pr, 0, half_dims),
                    in0=_ap(pr, 0, half_dims),
                    in1=_ap(pr, 128, half_dims),
                    op=ALU.add,
                )
                q_dims = [pr.ap[0], [1024, 16], [256, 4], [1, 64]]
                nc.vector.tensor_tensor(
                    out=_ap(pr, 0, q_dims),
                    in0=_ap(pr, 0, q_dims),
                    in1=_ap(pr, 64, q_dims),
                    op=ALU.add,
                )
                o_dims = [pr.ap[0], [1024, 16], [256, 4], [1, 32]]
                nc.vector.tensor_tensor(
                    out=_ap(pr, 0, o_dims),
                    in0=_ap(pr, 0, o_dims),
                    in1=_ap(pr, 32, o_dims),
                    op=ALU.add,
                )
                if stage < 2:
                    st["prod"] = prod
                    return
                s = psmall.tile([P, 512], F32, tag="s", bufs=2)
                sr = s[:]
                nc.vector.reduce_sum(
                    out=_ap(sr, 0, [sr.ap[0], [32, 16], [8, 4], [1, 8]]),
                    in_=_ap(pr, 0, [pr.ap[0], [1024, 16], [256, 4], [1, 8], [8, 4]]),
                    axis=AX.X,
                )
                # exp in place (same tile, cross-iteration lifetime via bufs=2)
                nc.scalar.activation(
                    out=s[:], in_=s[:], func=ACT.Exp, bias=0.0, scale=INV_SQRT_D
                )
                st["prod"] = prod
                st["e"] = s

            def phase_b(i):
                """softmax tail, AV, residual tree, stats (DVE); LN tail."""
                st = state[i]
                if stage < 3:
                    return
                e = st["e"]
                er = e[:]
                kr = st["kb"][:]
                qbr = st["qb"][:]
                z = psmall.tile([P, 128], F32, tag="z", bufs=1)
                nc.vector.reduce_sum(
                    out=z[:],
                    in_=_ap(er, 0, [er.ap[0], [32, 16], [1, 8], [8, 4]]),
                    axis=AX.X,
                )
                rz = psmall.tile([P, 128], F32, tag="rz", bufs=1)
                nc.vector.reciprocal(out=rz[:], in_=z[:])
                pn = psmall.tile([P, 512], BF16, tag="pn", bufs=1)
                rzr = rz[:]
                nc.vector.tensor_tensor(
                    out=pn[:],
                    in0=e[:],
                    in1=_ap(rzr, 0, [rzr.ap[0], [8, 16], [0, 4], [1, 8]]),
                    op=ALU.mult,
                )
                if stage < 4:
                    return
                pnr = pn[:]
                pv = pbig.tile([P, FP], BF16, tag="big")
                pvr = pv[:]
                nc.vector.tensor_tensor(
                    out=_ap(pvr, 0, [pvr.ap[0], [1, FP]]),
                    in0=_ap(pnr, 0, [pnr.ap[0], [32, 16], [8, 4], [0, D], [1, 8]]),
                    in1=_ap(kr, 0, [kr.ap[0], [NTOK * C, 4], [0, 4], [C, 4], [1, C]]),
                    op=ALU.mult,
                )
                # k-sum tree + residual, all bf16 2x, output stays permuted
                kk = pkk.tile([P, 2 * FQ], BF16, tag="kk")
                kkr = kk[:]
                qk_dims = [pvr.ap[0], [1024, 16], [1, 256]]
                nc.vector.tensor_tensor(
                    out=_ap(kkr, 0, [kkr.ap[0], [1, FQ]]),
                    in0=_ap(pvr, 0 * C, qk_dims),
                    in1=_ap(pvr, 1 * C, qk_dims),
                    op=ALU.add,
                )
                nc.vector.tensor_tensor(
                    out=_ap(kkr, FQ, [kkr.ap[0], [1, FQ]]),
                    in0=_ap(pvr, 2 * C, qk_dims),
                    in1=_ap(pvr, 3 * C, qk_dims),
                    op=ALU.add,
                )
                xrsq = pxr.tile([P, 2 * FQ], BF16, tag="xrsq")
                xr = xrsq[:]
                nc.vector.tensor_tensor(
                    out=_ap(xr, 0, [xr.ap[0], [1, FQ]]),
                    in0=_ap(kkr, 0, [kkr.ap[0], [1, FQ]]),
                    in1=_ap(kkr, FQ, [kkr.ap[0], [1, FQ]]),
                    op=ALU.add,
                )
                nc.vector.tensor_tensor(
                    out=_ap(xr, 0, [xr.ap[0], [1, FQ]]),
                    in0=_ap(xr, 0, [xr.ap[0], [1, FQ]]),
                    in1=_ap(qbr, 0, [qbr.ap[0], [1, FQ]]),
                    op=ALU.add,
                )
                st["xrsq"] = xrsq
                if stage < 5:
                    return
                # sq on the idle ACT engine; phase_a(i) hides its latency
                nc.scalar.activation(
                    out=_ap(xr, FQ, [xr.ap[0], [1, FQ]]),
                    in_=_ap(xr, 0, [xr.ap[0], [1, FQ]]),
                    func=ACT.Square, bias=0.0, scale=1.0,
                )

            def phase_b2(i):
                """stats + LN tail + normalize + store for tile i."""
                st = state[i]
                if stage < 5 or "xrsq" not in st:
                    del state[i]
                    return
                xr = st["xrsq"][:]
                sth = pkk.tile([P, FQ], BF16, tag="kk")
                str0 = sth[:]
                h_dims = [xr.ap[0], [C, 32], [1, 128]]
                nc.vector.tensor_tensor(
                    out=_ap(str0, 0, [str0.ap[0], [1, FQ]]),
                    in0=_ap(xr, 0, h_dims),
                    in1=_ap(xr, 128, h_dims),
                    op=ALU.add,
                )
                h2_dims = [str0.ap[0], [128, 32], [1, 64]]
                nc.vector.tensor_tensor(
                    out=_ap(str0, 0, h2_dims),
                    in0=_ap(str0, 0, h2_dims),
                    in1=_ap(str0, 64, h2_dims),
                    op=ALU.add,
                )
                stats = psmall.tile([P, 32], F32, tag="stats", bufs=1)
                str_ = stats[:]
                nc.vector.reduce_sum(
                    out=_ap(str_, 0, [str_.ap[0], [1, 32]]),
                    in_=_ap(str0, 0, [str0.ap[0], [128, 32], [1, 64]]),
                    axis=AX.X,
                )
                ms = psmall.tile([P, 32], F32, tag="ms", bufs=2)
                nc.vector.tensor_scalar(
                    out=ms[:], in0=stats[:], scalar1=1.0 / C, scalar2=None,
                    op0=ALU.mult,
                )
                musq = psmall.tile([P, 16], F32, tag="musq", bufs=1)
                nc.vector.tensor_tensor(
                    out=musq[:], in0=ms[:, :16], in1=ms[:, :16], op=ALU.mult
                )
                vpe = psmall.tile([P, 16], F32, tag="vpe", bufs=1)
                nc.vector.tensor_tensor(
                    out=vpe[:], in0=ms[:, 16:], in1=musq[:], op=ALU.subtract
                )
                lnv = psmall.tile([P, 16], F32, tag="lnv", bufs=1)
                nc.scalar.activation(
                    out=lnv[:], in_=vpe[:], func=ACT.Ln, bias=epst[:], scale=1.0
                )
                rs = psmall.tile([P, 16], F32, tag="rs", bufs=2)
                nc.scalar.activation(
                    out=rs[:], in_=lnv[:], func=ACT.Exp, bias=0.0, scale=-0.5
                )
                # normalize on Pool; t1a un-permutes (d,h)->true c (3 free dims)
                t1 = pnrm.tile([P, FQ], F32, tag="t1")
                t1r = t1[:]
                msr = ms[:]
                nc.gpsimd.tensor_tensor(
                    out=_ap(t1r, 0, [t1r.ap[0], [C, 16], [1, D], [D, NH]]),
                    in0=_ap(xr, 0, [xr.ap[0], [1, FQ]]),
                    in1=_ap(msr, 0, [msr.ap[0], [1, 16], [0, C]]),
                    op=ALU.subtract,
                )
                rsr = rs[:]
                nc.gpsimd.tensor_tensor(
                    out=t1r,
                    in0=t1r,
                    in1=_ap(rsr, 0, [rsr.ap[0], [1, 16], [0, C]]),
                    op=ALU.mult,
                )
                t3 = pnrm.tile([P, FQ], F32, tag="t3")
                wr = wcat[:]
                nc.gpsimd.tensor_tensor(
                    out=t3[:],
                    in0=t1r,
                    in1=_ap(wr, 0, [wr.ap[0], [C, 4], [0, 4], [1, C]]),
                    op=ALU.mult,
                )
                xout = pout.tile([P, NTOK * NX * C], F32, tag="xout")
                xor_ = xout[:]
                br = bcat[:]
                nc.gpsimd.tensor_tensor(
                    out=_ap(xor_, 0, [xor_.ap[0], [C, 4], [NX * C, 4], [1, C]]),
                    in0=t3[:],
                    in1=_ap(br, 0, [br.ap[0], [C, 4], [0, 4], [1, C]]),
                    op=ALU.add,
                )
                if stage >= 6:
                    t = i % NTILES
                    for qh in range(2):
                        dst = _ap(
                            out_ap,
                            (8 * t + qh) * W * NX * C,
                            [[2 * W * NX * C, 4], [2 * NX * C, NWI], [1, 2 * NX * C]],
                        )
                        src = _ap(
                            xor_, qh * 2 * NX * C, [xor_.ap[0], [1, 2 * NX * C]]
                        )
                        nc.sync.dma_start(out=dst, in_=src)
                del state[i]

            # ---- pipelined issue: B(i-1) first (big bufs=1), loads one ahead
            phase_load(0)
            for i in range(NT):
                if i >= 1:
                    phase_b(i - 1)
                if i + 1 < NT:
                    phase_load(i + 1)
                phase_a(i)
                if i >= 1:
                    phase_b2(i - 1)
            phase_b(NT - 1)
            phase_b2(NT - 1)
    return nc


def _split_multi_waits(nc):
    wid = 0
    for fn in nc.m.functions:
        for blk in fn.blocks:
            new_list = []
            changed = False
            for inst in blk.instructions:
                si = inst.sync_info
                if si is not None:
                    waits = list(si.on_wait)
                    if len(waits) > 1:
                        for w in waits[:-1]:
                            ev = mybir.InstEventSemaphore(
                                name=f"WSPLIT-{wid}", ins=[], outs=[]
                            )
                            wid += 1
                            ev.engine = inst.engine
                            ev.sync_info = bass_rust.SyncInfo(on_wait=[w], on_update=[])
                            new_list.append(ev)
                        inst.sync_info = bass_rust.SyncInfo(
                            on_wait=[waits[-1]], on_update=list(si.on_update)
                        )
                        changed = True
                new_list.append(inst)
            if changed:
                blk.instructions = new_list


_NC_CACHE = None


def _get_nc():
    global _NC_CACHE
    if _NC_CACHE is None:
        nc = build_kernel()
        _split_multi_waits(nc)
        _NC_CACHE = nc
    return _NC_CACHE


def kernel(**inputs) -> np.ndarray:
    nc = _get_nc()
    param_names = [f"ln{a + 1}_{s}" for a in range(4) for s in ("w", "b")]
    in_maps = []
    for ci in range(B):
        m = {
            name: np.ascontiguousarray(np.asarray(inputs[name])[ci], dtype=np.float32)
            for name in ("r", "g", "b", "ir")
        }
        for pnm in param_names:
            m[pnm] = np.ascontiguousarray(np.asarray(inputs[pnm]), dtype=np.float32)
        in_maps.append(m)
    res = run_bass_kernel_spmd(nc, in_maps, list(range(B)))
    return np.stack([res.results[ci]["out"] for ci in range(B)], axis=0)


if __name__ == "__main__":
    rng = np.random.default_rng(0)
    demo = {
        n: rng.standard_normal((B, H, W, C), dtype=np.float32)
        for n in ("r", "g", "b", "ir")
    }
    for a in range(4):
        demo[f"ln{a + 1}_w"] = rng.standard_normal(C).astype(np.float32)
        demo[f"ln{a + 1}_b"] = rng.standard_normal(C).astype(np.float32)
    o = kernel(**demo)
    print(o.shape, o.dtype)
